# revision 7
# baseline (speedup 1.0000x reference)
"""Distributed Trainium2 kernel for MQA causal attention (B=2, S=2048, D=2048,
N=8 query heads, K=1 kv head, H=256), sharded over 8 NeuronCores.

Sharding (SPMD-uniform, identical graph on every core):
  - Tensor-parallel over the 8 query heads: core n owns head n for BOTH batches.
  - KV projection data-parallel over the 4096 flattened tokens (512/core),
    followed by TWO 8-rank AllGathers (K first, then V) so the K half is on
    the wire while the V projection still runs.
  - After attention, enc is re-sharded head-split -> token-split via FOUR
    quarter AllToAlls (j half-H x h half-token), ordered j0h0, j1h0, j0h1,
    j1h1 so only the first quarter is exposed; the output projection runs as
    two token-half waves whose tiles complete staggered so the PSUM->SBUF
    copies and output stores hide under the remaining matmuls.

Schedule (v4):
  - Startup DMA priority: xkv + kvw stream first, rope-k tables next, the
    8x2MB x flood and the rest follow, so the KV projection (the AllGather's
    critical path) is never input-starved.
  - Attention is one flat software pipeline across all 8 blocks: logits+exp
    run 2 chunks ahead of the sums/encp accumulation, including across block
    boundaries.  finalize uses reciprocal_approx_fast (~5x faster than the
    exact DVE reciprocal) so PSUM banks free quickly at block boundaries.
"""

from contextlib import ExitStack

import numpy as np
import ml_dtypes

import concourse.bacc as bacc
import concourse.bass as bass
import concourse.mybir as mybir
import concourse.tile as tile
from concourse.bass_utils import run_bass_kernel_spmd

BF = mybir.dt.bfloat16
F32 = mybir.dt.float32

NCORES = 8
B, S, D, N, H = 2, 2048, 2048, 8, 256
BT = B * S            # 4096 flattened tokens
TSH = BT // NCORES    # 512 tokens per core (kv shard / output shard)
HH = H // 2           # 128, rope half
NQB = S // 512        # 4 query blocks of 512 per batch
AluOp = mybir.AluOpType


def _build():
    nc = bacc.Bacc(
        "TRN2",
        target_bir_lowering=False,
        debug=False,
        enable_asserts=True,
        num_devices=NCORES,
    )

    xTb = nc.dram_tensor("xTb", [8, 128, 8192], BF, kind="ExternalInput")
    xkv2 = nc.dram_tensor("xkv2", [128, 8192], BF, kind="ExternalInput")
    qw2 = nc.dram_tensor("qw2", [128, 4096], BF, kind="ExternalInput")
    kvw2 = nc.dram_tensor("kvw2", [128, 8192], BF, kind="ExternalInput")
    outw2 = nc.dram_tensor("outw2", [4, 128, 8192], BF, kind="ExternalInput")
    cosq = nc.dram_tensor("cosq", [HH, S], F32, kind="ExternalInput")
    sinq = nc.dram_tensor("sinq", [HH, S], F32, kind="ExternalInput")
    cosk = nc.dram_tensor("cosk", [HH, TSH], F32, kind="ExternalInput")
    sink = nc.dram_tensor("sink", [HH, TSH], F32, kind="ExternalInput")
    mask4 = nc.dram_tensor("mask4", [128, 2048], BF, kind="ExternalInput")
    out = nc.dram_tensor("out", [TSH, D], F32, kind="ExternalOutput")

    groups = [list(range(NCORES))]

    with tile.TileContext(nc) as tc, ExitStack() as es:
        consts = es.enter_context(tc.tile_pool(name="consts", bufs=1))

        def single(shape, dtype, name):
            return consts.tile(shape, dtype, name=name, tag=name)

        qw_sb = single([128, 16 * 256], BF, "qw_sb")
        cosq_sb = single([HH, S], F32, "cosq_sb")
        sinq_sb = single([HH, S], F32, "sinq_sb")
        cosk_sb = single([HH, TSH], F32, "cosk_sb")
        sink_sb = single([HH, TSH], F32, "sink_sb")
        mask_sb = single([128, 4 * 512], BF, "mask_sb")
        ones32 = single([128, 32], BF, "ones32")
        zeros_sq = single([128, 128], BF, "zeros_sq")
        sel4 = single([128, 128], BF, "sel4")
        qT_all = single([128, 2 * BT], BF, "qT_all")
        kT_sb = [single([128, BT], BF, f"kT{j}_sb") for j in range(2)]
        v_sb = single([128, (BT // 128) * 256], BF, "v_sb")
        enc_sb = [single([128, BT], BF, f"enc{j}_sb") for j in range(2)]

        psum = es.enter_context(tc.tile_pool(name="psum", bufs=8, space="PSUM"))
        bigp = es.enter_context(tc.tile_pool(name="bigp", bufs=1))
        xtp = es.enter_context(tc.tile_pool(name="xtp", bufs=3))
        tmpp = es.enter_context(tc.tile_pool(name="tmpp", bufs=6))
        stagep = es.enter_context(tc.tile_pool(name="stagep", bufs=1))
        ptp = es.enter_context(tc.tile_pool(name="ptp", bufs=8))
        rbp = es.enter_context(tc.tile_pool(name="rbp", bufs=3))
        osp = es.enter_context(tc.tile_pool(name="osp", bufs=4))
        dram = es.enter_context(tc.tile_pool(name="dram", bufs=1, space="DRAM"))

        kvw_sb = bigp.tile([128, 2 * 16 * 256], BF, name="kvw_sb", tag="big")

        kv_inK = dram.tile([128, 1024], BF, name="kv_inK", tag="kv_inK")
        kv_inV = dram.tile([128, 1024], BF, name="kv_inV", tag="kv_inV")
        kv_allK = dram.tile([NCORES, 128, 1024], BF, name="kv_allK",
                            tag="kv_allK", addr_space="Shared")
        kv_allV = dram.tile([NCORES, 128, 1024], BF, name="kv_allV",
                            tag="kv_allV", addr_space="Shared")
        # enc quarters: [j half-H][h half-token]
        enc_in = [[dram.tile([NCORES, 128, 256], BF, name=f"enc_in{j}{h}",
                             tag=f"enc_in{j}{h}") for h in range(2)]
                  for j in range(2)]
        enc_out = [[dram.tile([NCORES, 128, 256], BF, name=f"enc_out{j}{h}",
                              tag=f"enc_out{j}{h}") for h in range(2)]
                   for j in range(2)]

        nc.vector.memset(ones32[:], 1.0)
        nc.vector.memset(zeros_sq[:], 0.0)
        nc.vector.memset(sel4[:], 0.0)
        for m in range(4):
            nc.vector.memset(sel4[m * 32:m * 32 + 1, :], 1.0)

        # ---- KV projection over this core's 512-token shard ----
        # K matmuls first: rope-k and the K AllGather input store are on the
        # collective's critical path; V follows while rope-k runs on DVE.
        ktp = [psum.tile([128, 512], F32, name=f"ktp{j}", tag="bank")
               for j in range(2)]
        vp = [psum.tile([128, 512], F32, name=f"vp{i}", tag="bank")
              for i in range(2)]
        # kv chain split across sync (x) + scalar (weights) queues, tiny
        # first chunks so the first matmul fires right after the preamble;
        # rope tables load right behind the first K-weight chunk so rope-k
        # (the AllGather's critical path) never waits.
        xkt = xtp.tile([128, 16 * 512], BF, name="xkt", tag="xt")
        nc.sync.dma_start(xkt[:, 0:512], xkv2[:, 0:512])
        nc.scalar.dma_start(kvw_sb[:, 0:512], kvw2[:, 0:512])
        nc.scalar.dma_start(cosk_sb[:], cosk[:])
        nc.scalar.dma_start(sink_sb[:], sink[:])
        nc.sync.dma_start(xkt[:, 512:2048], xkv2[:, 512:2048])
        nc.scalar.dma_start(kvw_sb[:, 512:4096], kvw2[:, 512:4096])
        nc.sync.dma_start(xkt[:, 2048:4096], xkv2[:, 2048:4096])
        nc.sync.dma_start(xkt[:, 4096:8192], xkv2[:, 4096:8192])
        for c in range(2):
            v_sl = slice(4096 + c * 2048, 4096 + (c + 1) * 2048)
            nc.scalar.dma_start(kvw_sb[:, v_sl], kvw2[:, v_sl])
        nc.gpsimd.dma_start(qw_sb[:], qw2[:])

        for dc in range(16):
            st, sp = dc == 0, dc == 15
            xk = xkt[:, dc * 512:(dc + 1) * 512]
            for j in range(2):
                nc.tensor.matmul(
                    ktp[j][:],
                    lhsT=kvw_sb[:, dc * 256 + j * 128:dc * 256 + (j + 1) * 128],
                    rhs=xk,
                    start=st, stop=sp,
                )
        # rope k into packed store tiles (k0|k1), (v0|v1)
        kpack = stagep.tile([128, 1024], BF, name="kpack", tag="stage")
        vpack = stagep.tile([128, 1024], BF, name="vpack", tag="stage2")
        t_a = tmpp.tile([128, 512], F32, name="t_a", tag="tmp")
        t_b = tmpp.tile([128, 512], F32, name="t_b", tag="tmp")
        nc.vector.tensor_mul(t_a[:], ktp[0][:], cosk_sb[:])
        nc.vector.tensor_mul(t_b[:], ktp[1][:], sink_sb[:])
        nc.vector.tensor_sub(kpack[:, 0:512], t_a[:], t_b[:])
        t_c = tmpp.tile([128, 512], F32, name="t_c", tag="tmp")
        t_d = tmpp.tile([128, 512], F32, name="t_d", tag="tmp")
        nc.vector.tensor_mul(t_c[:], ktp[1][:], cosk_sb[:])
        nc.vector.tensor_mul(t_d[:], ktp[0][:], sink_sb[:])
        nc.vector.tensor_add(kpack[:, 512:1024], t_c[:], t_d[:])
        # K store + gather fire while the V projection still runs
        nc.gpsimd.dma_start(kv_inK[:], kpack[:])
        nc.gpsimd.collective_compute(
            "AllGather",
            AluOp.bypass,
            replica_groups=groups,
            ins=[kv_inK[:].opt()],
            outs=[kv_allK[:].opt()],
        )

        for dc in range(16):
            st, sp = dc == 0, dc == 15
            for i in range(4):
                nc.tensor.matmul(
                    vp[i // 2][:, (i % 2) * 256:(i % 2 + 1) * 256],
                    lhsT=xkt[:, dc * 512 + i * 128:dc * 512 + (i + 1) * 128],
                    rhs=kvw_sb[:, 4096 + dc * 256:4096 + (dc + 1) * 256],
                    start=(st and i % 2 == 0),
                    stop=(sp and i % 2 == 1),
                )
        for i in range(2):
            nc.vector.tensor_copy(vpack[:, i * 512:(i + 1) * 512], vp[i][:])
        nc.gpsimd.dma_start(kv_inV[:], vpack[:])
        nc.gpsimd.collective_compute(
            "AllGather",
            AluOp.bypass,
            replica_groups=groups,
            ins=[kv_inV[:].opt()],
            outs=[kv_allV[:].opt()],
        )

        # ---- q projection: x tiles stream on the sync queue behind xkt; the
        # rope tables weave in between the early tiles (needed by rope-q from
        # ~tb0's end).
        def xt_load(tb):
            xt = xtp.tile([128, 16 * 512], BF, name="xt", tag="xt")
            for hc in range(2):
                nc.sync.dma_start(xt[:, hc * 4096:(hc + 1) * 4096],
                                  xTb[tb, :, hc * 4096:(hc + 1) * 4096])
            return xt

        xts = {}
        xts[0] = xt_load(0)
        nc.sync.dma_start(cosq_sb[:], cosq[:])
        xts[1] = xt_load(1)
        nc.sync.dma_start(sinq_sb[:], sinq[:])
        nc.sync.dma_start(mask_sb[:], mask4[:])
        for tb in range(2, 8):
            xts[tb] = xt_load(tb)

        def load_kv_batch(b):
            # half-batch granularity: the first attention chunks only need
            # the first slots, so don't make them wait on the full 2MB.
            for hb in range(2):
                s0 = b * 4 + hb * 2
                sl = slice(s0, s0 + 2)
                c0 = s0 * 512
                nc.scalar.dma_start(
                    kT_sb[0][:, c0:c0 + 1024]
                    .rearrange("p (s t) -> p s t", s=2),
                    kv_allK[sl, :, 0:512].rearrange("s p t -> p s t"),
                )
                nc.gpsimd.dma_start(
                    kT_sb[1][:, c0:c0 + 1024]
                    .rearrange("p (s t) -> p s t", s=2),
                    kv_allK[sl, :, 512:1024].rearrange("s p t -> p s t"),
                )
                nc.scalar.dma_start(
                    v_sb[:, 2 * c0:2 * c0 + 2048]
                    .rearrange("p (s t) -> p s t", s=2),
                    kv_allV[sl, :, 0:1024].rearrange("s p t -> p s t"),
                )

        def qproj_batch(b):
            for tb in range(b * 4, b * 4 + 4):
                qtp = [psum.tile([128, 512], F32, name=f"qtp{j}", tag="bank")
                       for j in range(2)]
                xt = xts[tb]
                for dc in range(16):
                    for j in range(2):
                        nc.tensor.matmul(
                            qtp[j][:],
                            lhsT=qw_sb[:, dc * 256 + j * 128:
                                       dc * 256 + (j + 1) * 128],
                            rhs=xt[:, dc * 512:(dc + 1) * 512],
                            start=dc == 0, stop=dc == 15,
                        )
                cq = cosq_sb[:, (tb % 4) * 512:(tb % 4 + 1) * 512]
                sq = sinq_sb[:, (tb % 4) * 512:(tb % 4 + 1) * 512]
                u_a = tmpp.tile([128, 512], F32, name="u_a", tag="tmp")
                u_b = tmpp.tile([128, 512], F32, name="u_b", tag="tmp")
                nc.vector.tensor_mul(u_a[:], qtp[0][:], cq)
                nc.vector.tensor_mul(u_b[:], qtp[1][:], sq)
                nc.vector.tensor_sub(
                    qT_all[:, tb * 512:(tb + 1) * 512], u_a[:], u_b[:]
                )
                u_c = tmpp.tile([128, 512], F32, name="u_c", tag="tmp")
                u_d = tmpp.tile([128, 512], F32, name="u_d", tag="tmp")
                nc.vector.tensor_mul(u_c[:], qtp[1][:], cq)
                nc.vector.tensor_mul(u_d[:], qtp[0][:], sq)
                nc.vector.tensor_add(
                    qT_all[:, BT + tb * 512:BT + (tb + 1) * 512],
                    u_c[:], u_d[:]
                )

        qproj_batch(0)
        qproj_batch(1)
        load_kv_batch(0)
        load_kv_batch(1)
        # out-proj weights ride the scalar/sync queues BEHIND the kv loads /
        # x flood; they are only needed from ~200us so bandwidth is free.
        # Only 3 xtp bufs exist, so oww[3] loads once wave A releases one.
        oww = [None] * 4
        for db in range(3):
            eng = nc.scalar if db % 2 == 0 else nc.sync
            oww[db] = xtp.tile([128, 16 * 512], BF, name=f"oww{db}", tag="xt")
            eng.dma_start(oww[db][:], outw2[db])

        # ---- attention: flat software pipeline over all 8 blocks ----
        blocks = [(b, qb) for b in range(2) for qb in range(NQB)]
        jobs = []
        for bi, (b, qb) in enumerate(blocks):
            for ch in range(4 * (qb + 1)):
                jobs.append((bi, ch))

        state = {}

        def ensure_state(bi):
            if bi not in state:
                state[bi] = dict(
                    sums=psum.tile([128, 512], F32, name="sums", tag="bank"),
                    encp=[psum.tile([128, 512], F32, name=f"encp{j}",
                                    tag="bank") for j in range(2)],
                    stts={}, pts={},
                )

        def c0_of(bi, ch):
            nch = 4 * (blocks[bi][1] + 1)
            return max(ch - (nch - 4), 0) * 128

        def logits_exp(bi, ch):
            ensure_state(bi)
            b, qb = blocks[bi]
            nch = 4 * (qb + 1)
            c0 = c0_of(bi, ch)
            r = ch - (nch - 4)
            q0 = b * 2048 + qb * 512
            stt = psum.tile([128, 512], F32, name="stt", tag="bank")
            state[bi]["stts"][ch] = stt
            k0 = b * 2048 + ch * 128
            for j in range(2):
                nc.tensor.matmul(
                    stt[:, c0:],
                    lhsT=kT_sb[j][:, k0:k0 + 128],
                    rhs=qT_all[:, j * BT + q0 + c0:j * BT + q0 + 512],
                    start=j == 0, stop=j == 1,
                )
            pt = ptp.tile([128, 512], BF, name="pt", tag="pt")
            state[bi]["pts"][ch] = pt
            nc.scalar.activation(
                pt[:, c0:], stt[:, c0:],
                mybir.ActivationFunctionType.Exp,
            )
            if r >= 0:
                nc.vector.tensor_mul(
                    pt[:, c0:], pt[:, c0:],
                    mask_sb[:, r * 512 + c0:(r + 1) * 512],
                )

        def accum(bi, ch):
            b, qb = blocks[bi]
            nch = 4 * (qb + 1)
            c0 = c0_of(bi, ch)
            s = state[bi]
            pt = s["pts"][ch]
            s["stts"].pop(ch)
            first, last = ch == 0, ch == nch - 1
            if first and nch == 4:
                # qb==0 blocks: every chunk is diagonal, so parts of the
                # strip-sums bank are never written by the strip matmuls
                # below -- zero the whole bank first.
                nc.tensor.matmul(s["sums"][:], lhsT=zeros_sq[:], rhs=pt[:],
                                 start=True, stop=False)
            m = b * 16 + ch
            for j in range(2):
                nc.tensor.matmul(
                    s["encp"][j][:, c0:],
                    lhsT=v_sb[:, m * 256 + j * 128:m * 256 + (j + 1) * 128],
                    rhs=pt[:, c0:],
                    start=first, stop=last,
                )
            if ch % 4 == 3:
                # strip row-sums for the finished group of 4 chunks: four
                # 32-wide matmuls at distinct col-groups run concurrently
                for cc in range(ch - 3, ch + 1):
                    c0c = c0_of(bi, cc)
                    st = 32 * (cc % 4)
                    nc.tensor.matmul(
                        s["sums"][st:st + 32, c0c:],
                        lhsT=ones32[:],
                        rhs=s["pts"][cc][:, c0c:],
                        start=(cc < 4 and nch > 4), stop=cc >= nch - 4,
                        tile_position=(0, st),
                    )
                for cc in range(ch - 3, ch + 1):
                    s["pts"].pop(cc)
            if last:
                finalize(bi)

        def finalize(bi):
            b, qb = blocks[bi]
            q0 = b * 2048 + qb * 512
            s = state.pop(bi)
            ssb = rbp.tile([128, 512], BF, name="ssb", tag="ssb")
            nc.vector.tensor_copy(ssb[:], s["sums"][:])
            tot = psum.tile([128, 512], F32, name="tot", tag="bank")
            nc.tensor.matmul(tot[:], lhsT=sel4[:], rhs=ssb[:],
                             start=True, stop=True)
            rb_sb = rbp.tile([128, 512], F32, name="rb_sb", tag="rbs")
            nc.vector.reciprocal_approx_fast(out=rb_sb[:], in_=tot[:])
            for j in range(2):
                nc.vector.tensor_mul(
                    enc_sb[j][:, q0:q0 + 512], s["encp"][j][:], rb_sb[:],
                )
            blk = b * 4 + qb
            for j in range(2):
                eng = nc.scalar if j == 0 else nc.gpsimd
                for h in range(2):
                    eng.dma_start(
                        enc_in[j][h][blk],
                        enc_sb[j][:, q0 + h * 256:q0 + (h + 1) * 256],
                    )

        LOOK = 2
        for i in range(len(jobs) + LOOK):
            if i < len(jobs):
                logits_exp(*jobs[i])
            if i >= LOOK:
                accum(*jobs[i - LOOK])

        # ---- four quarter AllToAlls: head-split -> token-split ----
        # order j0h0, j0h1, j1h0, j1h1: wave A's j0 phases consume the first
        # two, so quarters 3 and 4 are hidden under wave-A matmuls.
        for (j, h) in ((0, 0), (0, 1), (1, 0), (1, 1)):
            nc.gpsimd.collective_compute(
                "AllToAll",
                AluOp.bypass,
                replica_groups=groups,
                ins=[enc_in[j][h][:].opt()],
                outs=[enc_out[j][h][:].opt()],
            )

        # encf layout: col block ((j*2+h)*8 + c)*256 + local = head c's
        # (j half-H, h half-token) enc for my 512 tokens
        encf_sb = bigp.tile([128, 2 * 8 * 512], BF, name="encf_sb", tag="big")

        def encf_load(j, h):
            # split in two so the first chunks land earlier
            for hr in range(2):
                eng = nc.sync if hr == 0 else nc.scalar
                base = (j * 2 + h) * 2048 + hr * 1024
                eng.dma_start(
                    encf_sb[:, base:base + 1024]
                    .rearrange("p (r t) -> p r t", r=4),
                    enc_out[j][h][hr * 4:(hr + 1) * 4]
                    .rearrange("r p t -> p r t"),
                )

        # ---- output projection: 2 db-waves, quarter-phased, staggered ----
        def finish_tile(ops, db, tt):
            h, t2 = tt // 2, tt % 2
            for c in range(8):
                nc.tensor.matmul(
                    ops[(db, tt)][:],
                    lhsT=encf_sb[:, ((2 + h) * 8 + c) * 256 + t2 * 128:
                                 ((2 + h) * 8 + c) * 256 + (t2 + 1) * 128],
                    rhs=oww[db][:, (8 + c) * 512:(8 + c + 1) * 512],
                    start=False, stop=c == 7,
                )
            o_sb = osp.tile([128, 512], F32, name="o_sb", tag="osb")
            nc.vector.tensor_copy(o_sb[:], ops[(db, tt)][:])
            eng = nc.scalar if (db + tt) % 2 == 0 else nc.sync
            eng.dma_start(
                out[tt * 128:(tt + 1) * 128, db * 512:(db + 1) * 512],
                o_sb[:],
            )

        def j0_phase(ops, dbs, h):
            for c in range(8):
                for db in dbs:
                    for t2 in range(2):
                        nc.tensor.matmul(
                            ops[(db, h * 2 + t2)][:],
                            lhsT=encf_sb[:, (h * 8 + c) * 256 + t2 * 128:
                                         (h * 8 + c) * 256 + (t2 + 1) * 128],
                            rhs=oww[db][:, c * 512:(c + 1) * 512],
                            start=c == 0, stop=False,
                        )

        # wave A (dbs 0,1): j0h0 -> j0h1 -> per-tile j1 completion
        opsA = {(db, tt): psum.tile([128, 512], F32, name=f"o{db}_{tt}",
                                    tag="bank")
                for db in (0, 1) for tt in range(4)}
        encf_load(0, 0)
        j0_phase(opsA, (0, 1), 0)
        encf_load(0, 1)
        j0_phase(opsA, (0, 1), 1)
        encf_load(1, 0)
        encf_load(1, 1)
        for db in (0, 1):
            for tt in range(4):
                finish_tile(opsA, db, tt)

        # wave B (dbs 2,3): all quarters landed; full per-tile chains
        oww[3] = xtp.tile([128, 16 * 512], BF, name="oww3", tag="xt")
        nc.sync.dma_start(oww[3][:], outw2[3])
        opsB = {(db, tt): psum.tile([128, 512], F32, name=f"o{db}_{tt}",
                                    tag="bank")
                for db in (2, 3) for tt in range(4)}
        for db in (2, 3):
            for tt in range(4):
                h, t2 = tt // 2, tt % 2
                for c in range(8):
                    nc.tensor.matmul(
                        opsB[(db, tt)][:],
                        lhsT=encf_sb[:, (h * 8 + c) * 256 + t2 * 128:
                                     (h * 8 + c) * 256 + (t2 + 1) * 128],
                        rhs=oww[db][:, c * 512:(c + 1) * 512],
                        start=c == 0, stop=False,
                    )
                finish_tile(opsB, db, tt)

    nc.compile()
    return nc


_NC_CACHE = None


def _get_nc():
    global _NC_CACHE
    if _NC_CACHE is None:
        _NC_CACHE = _build()
    return _NC_CACHE


def _rope_tables():
    freq_exp = (2.0 / H) * np.arange(HH, dtype=np.float32)
    timescale = (10000.0 ** freq_exp).astype(np.float32)  # [128]
    pos = np.arange(S, dtype=np.float32)
    rad = pos[None, :] / timescale[:, None]  # [128, 2048]
    return np.cos(rad).astype(np.float32), np.sin(rad).astype(np.float32)


def _mask4():
    kk = np.arange(128)[:, None, None]
    rr = np.arange(4)[None, :, None]
    tt = np.arange(512)[None, None, :]
    m = (kk + rr * 128 <= tt)  # [128, 4, 512]
    return np.ascontiguousarray(
        m.reshape(128, 2048).astype(ml_dtypes.bfloat16))


def _prepare_in_maps(x, q_w, kv_w, out_w):
    bf16 = ml_dtypes.bfloat16

    xb = np.asarray(x).reshape(BT, D).astype(bf16)  # [4096 tokens, 2048]
    xTb_h = np.ascontiguousarray(
        xb.reshape(8, 512, 16, 128).transpose(0, 3, 2, 1).reshape(8, 128, 8192)
    )
    qw_all = np.asarray(q_w).astype(bf16)  # [N, D, H]
    kvw_h = np.ascontiguousarray(
        np.asarray(kv_w)[:, 0].astype(bf16).reshape(2, 16, 128, 256)
        .transpose(2, 0, 1, 3).reshape(128, 8192)
    )
    # out-proj rhs chunks ordered (j, head): col block (j*8+h)*512 of db-slice
    # holds out_w rows [h, j*128:(j+1)*128] x D cols [db*512:(db+1)*512]
    outw_h = np.ascontiguousarray(
        np.asarray(out_w).astype(bf16).reshape(N, 2, 128, 4, 512)
        .transpose(3, 2, 1, 0, 4).reshape(4, 128, 8192)
    )
    cos_t, sin_t = _rope_tables()
    scale = np.float32(1.0 / np.sqrt(H))
    cosq_h = np.ascontiguousarray(cos_t * scale)
    sinq_h = np.ascontiguousarray(sin_t * scale)
    mask_h = _mask4()

    in_maps = []
    for n in range(NCORES):
        g0 = n * TSH
        posk = (np.arange(TSH) + g0) % S
        xkv_h = np.ascontiguousarray(
            xb[g0:g0 + TSH].reshape(512, 16, 128)
            .transpose(2, 1, 0).reshape(128, 8192)
        )
        qw_h = np.ascontiguousarray(
            qw_all[n].reshape(16, 128, 256).transpose(1, 0, 2)
            .reshape(128, 4096)
        )
        in_maps.append({
            "xTb": xTb_h,
            "xkv2": xkv_h,
            "qw2": qw_h,
            "kvw2": kvw_h,
            "outw2": outw_h,
            "cosq": cosq_h,
            "sinq": sinq_h,
            "cosk": np.ascontiguousarray(cos_t[:, posk]),
            "sink": np.ascontiguousarray(sin_t[:, posk]),
            "mask4": mask_h,
        })
    return in_maps


def _assemble_out(results):
    out = np.empty((B, S, D), dtype=np.float32)
    for n in range(NCORES):
        g0 = n * TSH
        out[g0 // S, g0 % S:g0 % S + TSH, :] = results[n]["out"]
    return out


def kernel(x, positions, attn_mask, q_w, kv_w, out_w):
    nc = _get_nc()
    in_maps = _prepare_in_maps(x, q_w, kv_w, out_w)
    res = run_bass_kernel_spmd(nc, in_maps, core_ids=list(range(NCORES)))
    return _assemble_out(res.results)
